# revision 28
# baseline (speedup 1.0000x reference)
"""Trainium2 Bass kernel for nn_MemTransformerLM (Transformer-XL layer).

Sharding (8 cores): batch (4) x head-half (2). Core c handles batch b = c//2
and heads [hh*8, hh*8+8), hh = c%2, for all 1024 queries. After o_proj a
2-rank bf16 ReduceScatter over core pairs (2b, 2b+1) splits tokens for the
FFN: even core keeps tokens [0,512), odd [512,1024).

All inputs are pre-cast/pre-tiled on the host into exact SBUF images (bf16),
so the kernel does straight DMA loads with no on-chip casts or layout
shuffles. o_proj, the ReduceScatter and the whole FFN run in transposed
layout (d on partitions), which eliminates every PE transpose; LayerNorm in
transposed layout uses a ones-matmul for the cross-partition mean/meansq
reduction.

Attention rel-shift: BD[i,j] = BD_raw[i, j-i+Q-1] is applied with a
"diagonal" SBUF->SBUF DMA (flat access pattern [[W-1,128],[1,N]]) that
accumulates the shifted BD window into the AC scores. The causal mask is
baked in by memsetting the out-of-range tail of each BD window to -30
before the shift, so exp() zeroes masked lanes without a mask pass.
"""

import contextlib

import numpy as np

import concourse.bass as bass
import concourse.bacc as bacc
import concourse.mybir as mybir
import concourse.tile as tile

F32 = mybir.dt.float32
BF16 = mybir.dt.bfloat16
FP8 = mybir.dt.float8e4
DR = mybir.MatmulPerfMode.DoubleRow
AF = mybir.ActivationFunctionType
ALU = mybir.AluOpType


class Cfg:
    D = 1024      # model dim
    NHC = 8       # heads per core
    DH = 64       # head dim
    KL = 2048     # key length
    Q = 1024      # query length
    DI = 4096     # ffn inner
    LN_EPS = 1e-5
    N_CORES = 8

    HD = property(lambda s: s.NHC * s.DH)       # head dims per core (512)
    SCALE = property(lambda s: 1.0 / (s.DH ** 0.5))
    M = property(lambda s: s.KL - s.Q)          # mem length
    NS = property(lambda s: s.Q // 128)         # q tiles (8)
    NJT = property(lambda s: s.KL // 128)       # key tiles (16)
    DPT = property(lambda s: s.D // 128)        # 8
    HPT = property(lambda s: s.HD // 128)       # 4
    NTT = property(lambda s: s.KL // 128)       # 16
    WB = property(lambda s: s.KL + 128)         # BD window buffer width
    TOKF = property(lambda s: s.Q // 2)         # ffn tokens per core (512)
    NM1 = property(lambda s: s.DI // 128)       # 32
    VW = property(lambda s: s.NHC * 65)         # v tile width (8 heads + ones)

    def jmax(self, s):
        # number of valid keys for q-tile s (128-granular)
        return min(self.KL, 128 * (s + 1) + self.M)

    def wstart(self, s):
        return self.Q - 128 * (s + 1)


def ts(i, n):
    return slice(i * n, (i + 1) * n)


def chunks(total, sz=512):
    return [(lo, min(total, lo + sz)) for lo in range(0, total, sz)]


def build_kernel(c: Cfg = None, collective=True):
    c = c or Cfg()
    nc = bacc.Bacc("TRN2", target_bir_lowering=False)

    io = {}
    def din(name, shape, dt=BF16):
        io[name] = nc.dram_tensor(name, shape, dt, kind="ExternalInput")
    # all pre-tiled SBUF images (see shard_inputs for layouts)
    din("xT", [128, c.DPT * c.KL], FP8)
    din("rT", [128, c.DPT * c.KL], FP8)
    din("wq", [128, c.DPT * c.HD], FP8)
    din("wk", [128, c.DPT * c.HD], FP8)
    din("wv", [128, c.DPT * c.HD], FP8)
    din("wr", [128, c.DPT * c.HD], FP8)
    din("wo", [128, c.HPT * c.D], FP8)
    din("w1", [128, c.NM1 * c.DPT * 128])
    din("w2", [128, c.DPT * c.NM1 * 128])
    din("wresT", [128, c.DPT * c.TOKF], F32)
    # consts: rwb(4) rrb(4) ffb1(32) ffb2(8) ln1g(8) ln1b(8) ln2g(8) ln2b(8)
    din("consts", [128, 80], F32)
    io["out"] = nc.dram_tensor("out", [128, c.DPT * c.TOKF], F32,
                               kind="ExternalOutput")
    io["rs_bin"] = nc.dram_tensor("rs_bin", [2 * c.D, c.TOKF], BF16)
    io["rs_bout"] = nc.dram_tensor("rs_bout", [c.D, c.TOKF], BF16)

    with tile.TileContext(nc) as tc:
        _body(tc, nc, c, io, collective=collective)
    nc.finalize()
    return nc


def _qslice(buf, c, hp, hr, s):
    """[64,128] lhsT slice for head (hp, hr) and q-tile s of a [128, HPT*Q] buf."""
    return buf[hr:hr + 64, hp * c.Q + s * 128: hp * c.Q + (s + 1) * 128]


def _body(tc, nc, c, io, collective=True):
    ctx = contextlib.ExitStack()
    rg = [[i, i + 1] for i in range(0, c.N_CORES, 2)]
    with ctx:
        small = ctx.enter_context(tc.tile_pool(name="small", bufs=4))
        keep = ctx.enter_context(tc.tile_pool(name="keep", bufs=1))
        consts = keep.tile([128, 80], F32, tag="consts")
        nc.sync.dma_start(out=consts[:], in_=io["consts"].ap().opt())
        rwb_s = consts[:, 0:4]
        rrb_s = consts[:, 4:8]
        fb1 = consts[:, 8:40]
        fb2 = consts[:, 40:48]
        lng = {"ln1g": consts[:, 48:56], "ln1b": consts[:, 56:64],
               "ln2g": consts[:, 64:72], "ln2b": consts[:, 72:80]}
        onesb = keep.tile([128, 128], BF16, tag="onesb")
        nc.vector.memset(onesb[:], 1.0)
        eps_t = keep.tile([128, 1], F32, tag="eps")
        nc.vector.memset(eps_t[:], c.LN_EPS)

        # ============ phase A: R^T -> rTp (rel-pos keys, windowed) ============
        atp = tc.alloc_tile_pool(name="atp", bufs=1)
        attnT = atp.tile([128, c.HPT * c.Q], FP8, tag="attnT")
        wo_t = atp.tile([128, c.HPT * c.D], FP8, tag="wo")
        nc.sync.dma_start(out=wo_t[:], in_=io["wo"].ap().opt())
        attk = tc.alloc_tile_pool(name="attk", bufs=1)
        rTp = attk.tile([128, c.HPT * c.WB], BF16, tag="rTp")
        kT = attk.tile([128, c.HPT * c.KL], BF16, tag="kT")
        vb = attk.tile([128, c.NTT * c.VW], BF16, tag="vb")
        rwq = attk.tile([128, c.HPT * c.Q], BF16, tag="rwq")
        rrq = attk.tile([128, c.HPT * c.Q], BF16, tag="rrq")

        inpX = tc.alloc_tile_pool(name="inpX", bufs=1)
        xT = inpX.tile([128, c.DPT * c.KL], FP8, tag="xT")
        nc.sync.dma_start(out=xT[:], in_=io["xT"].ap().opt())
        wq_t = inpX.tile([128, c.DPT * c.HD], FP8, tag="wq")
        nc.sync.dma_start(out=wq_t[:], in_=io["wq"].ap().opt())
        wk_t = inpX.tile([128, c.DPT * c.HD], FP8, tag="wk")
        nc.sync.dma_start(out=wk_t[:], in_=io["wk"].ap().opt())
        wv_t = inpX.tile([128, c.DPT * c.HD], FP8, tag="wv")
        nc.sync.dma_start(out=wv_t[:], in_=io["wv"].ap().opt())
        psF = tc.alloc_tile_pool(name="psF", bufs=4, space="PSUM")

        def ps_a():
            return psF.tile([128, 512], F32, tag="f", name="psf")

        with tc.tile_pool(name="inpR", bufs=1) as inpR:
            rT = inpR.tile([128, c.DPT * c.KL], FP8, tag="rT")
            nc.sync.dma_start(out=rT[:], in_=io["rT"].ap().opt())
            wr_t = inpR.tile([128, c.DPT * c.HD], FP8, tag="wr")
            nc.sync.dma_start(out=wr_t[:], in_=io["wr"].ap().opt())
            for m in range(c.HPT):
                nc.gpsimd.memset(rTp[:, m * c.WB + c.KL:(m + 1) * c.WB], 0.0)
                for lo, hi in chunks(c.KL):
                    ps = ps_a()
                    for kk in range(c.DPT // 2):
                        lhs = bass.AP(
                            tensor=wr_t.tensor,
                            offset=wr_t.offset + (m * 4 + kk) * 256,
                            ap=[[c.DPT * c.HD, 128], [128, 2], [1, 128]])
                        rhs = bass.AP(
                            tensor=rT.tensor,
                            offset=rT.offset + 2 * kk * c.KL + lo,
                            ap=[[c.DPT * c.KL, 128], [c.KL, 2], [1, hi - lo]])
                        nc.tensor.matmul(ps[:, 0:hi - lo], lhs, rhs, start=(kk == 0),
                                         stop=(kk == c.DPT // 2 - 1), perf_mode=DR)
                    nc.scalar.activation(
                        out=rTp[:, m * c.WB + lo: m * c.WB + hi],
                        in_=ps[:, 0:hi - lo], func=AF.Copy)
        # K^T [dh-pair part, keys] and Q^T with biases, per head pair
        if True:
            for m in range(c.HPT):
                for lo, hi in chunks(c.KL):
                    ps = ps_a()
                    for kk in range(c.DPT // 2):
                        lhs = bass.AP(
                            tensor=wk_t.tensor,
                            offset=wk_t.offset + (m * 4 + kk) * 256,
                            ap=[[c.DPT * c.HD, 128], [128, 2], [1, 128]])
                        rhs = bass.AP(
                            tensor=xT.tensor,
                            offset=xT.offset + 2 * kk * c.KL + lo,
                            ap=[[c.DPT * c.KL, 128], [c.KL, 2], [1, hi - lo]])
                        nc.tensor.matmul(ps[:, 0:hi - lo], lhs, rhs, start=(kk == 0),
                                         stop=(kk == c.DPT // 2 - 1), perf_mode=DR)
                    nc.scalar.activation(
                        out=kT[:, m * c.KL + lo: m * c.KL + hi],
                        in_=ps[:, 0:hi - lo], func=AF.Copy)
                for lo, hi in chunks(c.Q):
                    ps = ps_a()
                    for kk in range(c.DPT // 2):
                        lhs = bass.AP(
                            tensor=wq_t.tensor,
                            offset=wq_t.offset + (m * 4 + kk) * 256,
                            ap=[[c.DPT * c.HD, 128], [128, 2], [1, 128]])
                        rhs = bass.AP(
                            tensor=xT.tensor,
                            offset=xT.offset + 2 * kk * c.KL + c.M + lo,
                            ap=[[c.DPT * c.KL, 128], [c.KL, 2], [1, hi - lo]])
                        nc.tensor.matmul(ps[:, 0:hi - lo], lhs, rhs, start=(kk == 0),
                                         stop=(kk == c.DPT // 2 - 1), perf_mode=DR)
                    sl = slice(m * c.Q + lo, m * c.Q + hi)
                    nc.scalar.activation(out=rwq[:, sl], in_=ps[:, 0:hi - lo],
                                         func=AF.Identity, bias=rwb_s[:, m:m + 1])
                    nc.vector.tensor_scalar_add(out=rrq[:, sl], in0=ps[:, 0:hi - lo],
                                                scalar1=rrb_s[:, m:m + 1])
            # V natural [keys part, dh] (+ ones col per head)
            for m in range(c.NTT):
                for lo, hi in chunks(c.HD):
                    ps = ps_a()
                    for kk in range(c.DPT // 2):
                        lhs = bass.AP(
                            tensor=xT.tensor,
                            offset=xT.offset + 2 * kk * c.KL + m * 128,
                            ap=[[c.DPT * c.KL, 128], [c.KL, 2], [1, 128]])
                        rhs = bass.AP(
                            tensor=wv_t.tensor,
                            offset=wv_t.offset + kk * 2 * c.HD + lo,
                            ap=[[c.DPT * c.HD, 128], [c.HD, 2], [1, hi - lo]])
                        nc.tensor.matmul(ps[:, 0:hi - lo], lhs, rhs, start=(kk == 0),
                                         stop=(kk == c.DPT // 2 - 1), perf_mode=DR)
                    nheads = (hi - lo) // c.DH
                    dst = bass.AP(
                        tensor=vb.tensor,
                        offset=vb.offset + m * c.VW + (lo // c.DH) * 65,
                        ap=[[c.NTT * c.VW, 128], [65, nheads], [1, c.DH]])
                    nc.vector.tensor_copy(out=dst, in_=ps[:, 0:hi - lo])
                ones = bass.AP(
                    tensor=vb.tensor, offset=vb.offset + m * c.VW + c.DH,
                    ap=[[c.NTT * c.VW, 128], [65, c.NHC], [1, 1]])
                nc.vector.memset(ones, 1.0)
        inpX.release()
        psF.release()

        # ============ phase C: attention ============
        def attn_v(h, pT, ci):
            hp, hr = h // 2, (h % 2) * 64
            lo, hi = ci * 512, (ci + 1) * 512
            ps = psVp.tile([65, 512], F32, tag="v")
            s0 = 4 * ci  # first q-tile of this chunk
            njt_max = c.jmax(s0 + 3) // 128
            for jt in range(njt_max):
                # q-tiles s with jmax(s) > jt*128 form a suffix
                sl0 = s0
                while c.jmax(sl0) <= jt * 128:
                    sl0 += 1
                qa = lo + (sl0 - s0) * 128
                nc.tensor.matmul(
                    ps[0:65, qa - lo:hi - lo],
                    vb[:, jt * c.VW + h * 65: jt * c.VW + h * 65 + 65],
                    pT[:, jt * c.Q + qa: jt * c.Q + hi],
                    start=(jt == 0), stop=(jt == njt_max - 1))
            rd = small.tile([1, 512], F32, tag="rd")
            nc.vector.reciprocal(out=rd[0:1, 0:hi - lo], in_=ps[64:65, 0:hi - lo])
            rdb = small.tile([64, 512], F32, tag="rdb")
            src_b = bass.AP(tensor=rd.tensor, offset=rd.offset,
                            ap=[[512, 1], [0, 64], [1, hi - lo]])
            nc.sync.dma_start(out=rdb[:, 0:hi - lo], in_=src_b)
            nc.vector.tensor_tensor(
                out=attnT[hr:hr + 64, hp * c.Q + lo: hp * c.Q + hi],
                in0=ps[0:64, 0:hi - lo], in1=rdb[:, 0:hi - lo], op=ALU.mult)

        with tc.tile_pool(name="score", bufs=6) as score, \
             tc.tile_pool(name="scoreW", bufs=4) as scoreW, \
             tc.tile_pool(name="pTp", bufs=2) as pTp, \
             tc.tile_pool(name="psAC", bufs=3, space="PSUM") as psAC, \
             tc.tile_pool(name="psBD", bufs=3, space="PSUM") as psBD, \
             tc.tile_pool(name="psVp", bufs=2, space="PSUM") as psVp:
            evac_ci = [0]
            for h in range(c.NHC):
                hp, hr = h // 2, (h % 2) * 64
                pT = pTp.tile([128, c.NJT * c.Q], BF16, tag="pT")
                for s in range(c.NS):
                    jm, wst = c.jmax(s), c.wstart(s)
                    wneed = jm + 128
                    wreal = min(wneed, c.KL - wst)
                    # BD_raw window [128 q, wreal] (scaled bf16); width ==
                    # pitch must equal wneed for the flat diagonal AP below.
                    bdw = scoreW.tile([128, wneed], BF16, tag="bdw")
                    for lo, hi in chunks(wreal):
                        ps = psBD.tile([128, 512], F32, tag="c", name="psc")
                        nc.tensor.matmul(
                            ps[:, 0:hi - lo], _qslice(rrq, c, hp, hr, s),
                            rTp[hr:hr + 64, hp * c.WB + wst + lo: hp * c.WB + wst + hi],
                            start=True, stop=True)
                        evac_ci[0] += 1
                        if evac_ci[0] % 3 == 0:
                            nc.scalar.activation(
                                out=bdw[:, lo:hi], in_=ps[:, 0:hi - lo],
                                func=AF.Copy, scale=float(c.SCALE))
                        else:
                            nc.vector.tensor_scalar_mul(
                                out=bdw[:, lo:hi], in0=ps[:, 0:hi - lo],
                                scalar1=float(c.SCALE))
                    if wneed > wreal:
                        nc.gpsimd.memset(bdw[:, wreal:wneed], -30.0)
                    # AC [128 q, jm] (scaled bf16)
                    sb = score.tile([128, c.KL], BF16, tag="sb")
                    for lo, hi in chunks(jm):
                        ps = psAC.tile([128, 512], F32, tag="b", name="psb")
                        nc.tensor.matmul(
                            ps[:, 0:hi - lo], _qslice(rwq, c, hp, hr, s),
                            kT[hr:hr + 64, hp * c.KL + lo: hp * c.KL + hi],
                            start=True, stop=True)
                        evac_ci[0] += 1
                        if evac_ci[0] % 3 == 0:
                            nc.scalar.activation(
                                out=sb[:, lo:hi], in_=ps[:, 0:hi - lo],
                                func=AF.Copy, scale=float(c.SCALE))
                        else:
                            nc.vector.tensor_scalar_mul(
                                out=sb[:, lo:hi], in0=ps[:, 0:hi - lo],
                                scalar1=float(c.SCALE))
                    # shifted BD accumulated onto AC via diagonal DMA
                    diag = bass.AP(tensor=bdw.tensor, offset=bdw.offset + 127,
                                   ap=[[wneed - 1, 128], [1, jm]])
                    nc.gpsimd.dma_start(out=sb[:, 0:jm], in_=diag, accum_op=ALU.add)
                    nc.scalar.activation(out=sb[:, 0:jm], in_=sb[:, 0:jm],
                                         func=AF.Exp)
                    dstap = bass.AP(
                        tensor=pT.tensor, offset=pT.offset + s * 128,
                        ap=[[c.NJT * c.Q, 128], [c.Q, jm // 128], [1, 128]])
                    nc.scalar.dma_start(out=dstap, in_=sb[:, 0:jm], transpose=True)
                    if s == 3:
                        attn_v(h, pT, 0)
                attn_v(h, pT, 1)

        attk.release()

        # ============ phase D: o_proj (transposed) -> ReduceScatter ============
        with tc.tile_pool(name="phD", bufs=1) as phD, \
             tc.tile_pool(name="stageD", bufs=3) as stage, \
             tc.tile_pool(name="psD", bufs=3, space="PSUM") as psD:
            for m in range(c.DPT):
                for half, (lo, hi) in enumerate(chunks(c.Q)):
                    ps = psD.tile([128, 512], F32, tag="d", name="psd")
                    for kk in range(c.HPT // 2):
                        lhs = bass.AP(
                            tensor=wo_t.tensor,
                            offset=wo_t.offset + (m * 2 + kk) * 256,
                            ap=[[c.HPT * c.D, 128], [128, 2], [1, 128]])
                        rhs = bass.AP(
                            tensor=attnT.tensor,
                            offset=attnT.offset + kk * 2 * c.Q + lo,
                            ap=[[c.HPT * c.Q, 128], [c.Q, 2], [1, hi - lo]])
                        nc.tensor.matmul(ps[:, 0:hi - lo], lhs, rhs, start=(kk == 0),
                                         stop=(kk == c.HPT // 2 - 1), perf_mode=DR)
                    ob = stage.tile([128, 512], BF16, tag="oT")
                    nc.vector.tensor_copy(out=ob[:], in_=ps[:, 0:hi - lo])
                    nc.sync.dma_start(
                        out=io["rs_bin"][half * c.D + m * 128: half * c.D + (m + 1) * 128, :],
                        in_=ob[:])
        if collective:
            nc.gpsimd.collective_compute(
                "ReduceScatter", ALU.add, replica_groups=rg,
                ins=[io["rs_bin"].ap().opt()], outs=[io["rs_bout"].ap().opt()])
        else:
            # timeline-sim variant: plain copy standing in for the pair RS
            nc.sync.dma_start(out=io["rs_bout"].ap().opt(),
                              in_=io["rs_bin"].ap()[0:c.D, :].opt())
        atp.release()

        # ============ phase E: LN1 + FFN + LN2 (all transposed) ============
        def layer_norm_T(src_f32, src_b16, g, b, outs):
            """LayerNorm over d (partitions x DPT tiles), transposed layout.

            src_f32(k): AP [128, TOKF] f32 value source per d-tile.
            src_b16(k): AP [128, TOKF] bf16 copy (stats matmul operand).
            outs: list of (dst_fn, ) writers; each dst_fn(k) -> AP.
            """
            x2 = ffn.tile([128, c.DPT * c.TOKF], BF16, tag="x2")
            for k in range(c.DPT):
                nc.scalar.activation(out=x2[:, ts(k, c.TOKF)], in_=src_f32(k),
                                     func=AF.Square)
            psS = psE.tile([128, 512], F32, tag="s", name="psS")
            for k in range(c.DPT):
                nc.tensor.matmul(psS[:], onesb[:], src_b16(k),
                                 start=(k == 0), stop=(k == c.DPT - 1))
            psQ = psE.tile([128, 512], F32, tag="q", name="psQ")
            for k in range(c.DPT):
                nc.tensor.matmul(psQ[:], onesb[:], x2[:, ts(k, c.TOKF)],
                                 start=(k == 0), stop=(k == c.DPT - 1))
            mu = ffn.tile([128, c.TOKF], F32, tag="mu")
            nc.vector.tensor_scalar_mul(out=mu[:], in0=psS[:], scalar1=1.0 / c.D)
            # var = E[x^2] - mu^2 ; rstd = 1/sqrt(var + eps)
            var = ffn.tile([128, c.TOKF], F32, tag="var")
            nc.vector.tensor_scalar_mul(out=var[:], in0=psQ[:], scalar1=1.0 / c.D)
            mu2 = ffn.tile([128, c.TOKF], F32, tag="mu2")
            nc.vector.tensor_tensor(out=mu2[:], in0=mu[:], in1=mu[:], op=ALU.mult)
            nc.vector.tensor_tensor(out=var[:], in0=var[:], in1=mu2[:], op=ALU.subtract)
            nc.scalar.activation(out=var[:], in_=var[:], func=AF.Sqrt,
                                 bias=eps_t[:], scale=1.0)
            nc.vector.reciprocal(out=var[:], in_=var[:])
            nrm = ffn.tile([128, c.DPT * c.TOKF], F32, tag="nrm")
            for k in range(c.DPT):
                od = nrm[:, ts(k, c.TOKF)]
                nc.vector.tensor_tensor(out=od, in0=src_f32(k), in1=mu[:],
                                        op=ALU.subtract)
                nc.vector.tensor_tensor(out=od, in0=od, in1=var[:], op=ALU.mult)
                for dst_fn in outs:
                    nc.vector.tensor_scalar(out=dst_fn(k), in0=od,
                                            scalar1=g[:, k:k + 1], scalar2=b[:, k:k + 1],
                                            op0=ALU.mult, op1=ALU.add)

        with tc.tile_pool(name="ffn", bufs=1) as ffn, \
             tc.tile_pool(name="stageE", bufs=3) as stage, \
             tc.tile_pool(name="psE", bufs=2, space="PSUM") as psE, \
             tc.tile_pool(name="psE2", bufs=3, space="PSUM") as psE2:
            zr = ffn.tile([128, c.DPT * c.TOKF], BF16, tag="zr")
            srcap = bass.AP(tensor=io["rs_bout"].ap().tensor, offset=0,
                            ap=[[c.TOKF, 128], [128 * c.TOKF, c.DPT], [1, c.TOKF]])
            nc.sync.dma_start(out=zr[:], in_=srcap)
            wrT = ffn.tile([128, c.DPT * c.TOKF], F32, tag="wrT")
            nc.sync.dma_start(out=wrT[:], in_=io["wresT"].ap().opt())
            zT = ffn.tile([128, c.DPT * c.TOKF], F32, tag="zT")
            nc.vector.tensor_tensor(out=zT[:], in0=zr[:], in1=wrT[:], op=ALU.add)
            zb = ffn.tile([128, c.DPT * c.TOKF], BF16, tag="zb")
            nc.vector.tensor_copy(out=zb[:], in_=zT[:])
            ln1 = ffn.tile([128, c.DPT * c.TOKF], BF16, tag="ln1")
            layer_norm_T(lambda k: zT[:, ts(k, c.TOKF)],
                         lambda k: zb[:, ts(k, c.TOKF)],
                         lng["ln1g"], lng["ln1b"],
                         [lambda k: ln1[:, ts(k, c.TOKF)]])
            # FFN1: h^T[m1] = relu(sum_k w1[m1,k]^T ln1[k] + b1)
            hT = ffn.tile([128, c.NM1 * c.TOKF], BF16, tag="hT")
            for m1 in range(c.NM1):
                w1m = stage.tile([128, c.DPT * 128], BF16, tag="w1m")
                nc.sync.dma_start(
                    out=w1m[:],
                    in_=io["w1"].ap()[:, m1 * c.DPT * 128:(m1 + 1) * c.DPT * 128].opt())
                ps = psE2.tile([128, 512], F32, tag="e", name="pse")
                for k in range(c.DPT):
                    nc.tensor.matmul(
                        ps[:], w1m[:, ts(k, 128)], ln1[:, ts(k, c.TOKF)],
                        start=(k == 0), stop=(k == c.DPT - 1))
                nc.scalar.activation(out=hT[:, ts(m1, c.TOKF)], in_=ps[:],
                                     func=AF.Relu, bias=fb1[:, m1:m1 + 1])
            # FFN2 + residual
            o2 = ffn.tile([128, c.DPT * c.TOKF], F32, tag="o2")
            o2b = ffn.tile([128, c.DPT * c.TOKF], BF16, tag="o2b")
            for m in range(c.DPT):
                w2m = stage.tile([128, c.NM1 * 128], BF16, tag="w2m")
                nc.sync.dma_start(
                    out=w2m[:],
                    in_=io["w2"].ap()[:, m * c.NM1 * 128:(m + 1) * c.NM1 * 128].opt())
                ps = psE2.tile([128, 512], F32, tag="e", name="pse")
                for k in range(c.NM1):
                    nc.tensor.matmul(
                        ps[:], w2m[:, ts(k, 128)], hT[:, ts(k, c.TOKF)],
                        start=(k == 0), stop=(k == c.NM1 - 1))
                nc.scalar.activation(out=o2[:, ts(m, c.TOKF)], in_=ps[:],
                                     func=AF.Identity, bias=fb2[:, m:m + 1])
                nc.vector.tensor_add(out=o2[:, ts(m, c.TOKF)],
                                     in0=o2[:, ts(m, c.TOKF)],
                                     in1=ln1[:, ts(m, c.TOKF)])
                nc.vector.tensor_copy(out=o2b[:, ts(m, c.TOKF)],
                                      in_=o2[:, ts(m, c.TOKF)])
            fin = ffn.tile([128, c.DPT * c.TOKF], F32, tag="fin")
            layer_norm_T(lambda k: o2[:, ts(k, c.TOKF)],
                         lambda k: o2b[:, ts(k, c.TOKF)],
                         lng["ln2g"], lng["ln2b"],
                         [lambda k: fin[:, ts(k, c.TOKF)]])
            for k in range(c.DPT):
                nc.sync.dma_start(
                    out=io["out"].ap()[:, ts(k, c.TOKF)].opt(),
                    in_=fin[:, ts(k, c.TOKF)])


# ============================================================
# host-side sharding + entry point
# ============================================================

def shard_inputs(inputs, c: Cfg = None):
    import ml_dtypes
    c = c or Cfg()
    bf16 = ml_dtypes.bfloat16
    w = np.asarray(inputs["w"], np.float32)            # [Q, B, D]
    r = np.asarray(inputs["r"], np.float32)            # [KL, 1, D]
    mems = np.asarray(inputs["mems"], np.float32)      # [M, B, D]
    qkv_w = np.asarray(inputs["qkv_w"], np.float32)    # [D, 3*NHD]
    r_net_w = np.asarray(inputs["r_net_w"], np.float32)
    o_w = np.asarray(inputs["o_w"], np.float32)        # [NHD, D]
    r_w_bias = np.asarray(inputs["r_w_bias"], np.float32).reshape(-1)
    r_r_bias = np.asarray(inputs["r_r_bias"], np.float32).reshape(-1)
    ff_w1 = np.asarray(inputs["ff_w1"], np.float32)    # [D, DI]
    ff_w2 = np.asarray(inputs["ff_w2"], np.float32)    # [DI, D]
    NHD = qkv_w.shape[1] // 3

    def timg(x):
        # x [KD, C] with KD = k*128+p -> [128, nk*C]; img[p, k*C + j] = x[k*128+p, j]
        KD, C = x.shape
        nk = KD // 128
        return np.ascontiguousarray(
            x.reshape(nk, 128, C).transpose(1, 0, 2).reshape(128, nk * C)
        ).astype(bf16)

    def pcol(x):
        # [KD] -> [128, KD//128] per-partition scalar image
        KD = x.shape[0]
        return x.reshape(KD // 128, 128).T

    fp8 = ml_dtypes.float8_e4m3
    # w1 image: [128, (m1*DPT + k)*128 + cc] = ff_w1[k*128+p, m1*128+cc]
    w1i = ff_w1.reshape(c.DPT, 128, c.NM1, 128).transpose(1, 2, 0, 3) \
        .reshape(128, c.NM1 * c.DPT * 128)
    # w2 image: [128, (m*NM1 + k)*128 + cc] = ff_w2[k*128+p, m*128+cc]
    w2i = ff_w2.reshape(c.NM1, 128, c.DPT, 128).transpose(1, 2, 0, 3) \
        .reshape(128, c.DPT * c.NM1 * 128)
    w1i = np.ascontiguousarray(w1i).astype(bf16)
    w2i = np.ascontiguousarray(w2i).astype(bf16)

    def drw(x, nm):
        # DR lhsT image for [KD, MD] weight: [128, ((m*KK+kk)*2+i)*128+mc]
        KD, MD = x.shape
        kkn = KD // 256
        return np.ascontiguousarray(
            x.reshape(kkn, 2, 128, MD // 128, 128).transpose(2, 3, 0, 1, 4)
            .reshape(128, KD // 128 * MD)).astype(fp8)
    rTi = timg(np.ascontiguousarray(r[:, 0, :].T)).astype(fp8)

    in_maps = []
    for core in range(c.N_CORES):
        b, hh = core // 2, core % 2
        hsl = slice(hh * c.HD, (hh + 1) * c.HD)
        cat = np.concatenate([mems[:, b, :], w[:, b, :]], axis=0)  # [KL, D]
        consts = np.zeros((128, 80), np.float32)
        consts[:, 0:4] = pcol(r_w_bias[hsl])
        consts[:, 4:8] = pcol(r_r_bias[hsl])
        consts[:, 8:40] = pcol(np.asarray(inputs["ff_b1"], np.float32))
        consts[:, 40:48] = pcol(np.asarray(inputs["ff_b2"], np.float32))
        consts[:, 48:56] = pcol(np.asarray(inputs["ln1_g"], np.float32))
        consts[:, 56:64] = pcol(np.asarray(inputs["ln1_b"], np.float32))
        consts[:, 64:72] = pcol(np.asarray(inputs["ln2_g"], np.float32))
        consts[:, 72:80] = pcol(np.asarray(inputs["ln2_b"], np.float32))
        tok = slice(hh * c.TOKF, (hh + 1) * c.TOKF)
        wv_sl = qkv_w[:, 2 * NHD + hh * c.HD: 2 * NHD + (hh + 1) * c.HD]
        wvi = np.ascontiguousarray(
            wv_sl.reshape(c.DPT // 2, 2, 128, c.HD).transpose(2, 0, 1, 3)
            .reshape(128, c.DPT * c.HD)).astype(fp8)
        in_maps.append({
            "xT": timg(np.ascontiguousarray(cat.T)).astype(fp8),
            "rT": rTi,
            "wq": drw(qkv_w[:, 0 * NHD + hh * c.HD: 0 * NHD + (hh + 1) * c.HD], "wq"),
            "wk": drw(qkv_w[:, 1 * NHD + hh * c.HD: 1 * NHD + (hh + 1) * c.HD], "wk"),
            "wv": wvi,
            "wr": drw(r_net_w[:, hsl], "wr"),
            "wo": drw(o_w[hsl, :], "wo"),
            "w1": w1i,
            "w2": w2i,
            "wresT": np.ascontiguousarray(
                w[tok, b, :].T.reshape(c.DPT, 128, c.TOKF)
                .transpose(1, 0, 2).reshape(128, c.DPT * c.TOKF)),
            "consts": consts,
        })
    return in_maps


def unshard_output(results, inputs, c: Cfg = None):
    c = c or Cfg()
    w = np.asarray(inputs["w"])
    Q, B, D = w.shape
    out = np.zeros((Q, B, D), np.float32)
    for core in range(c.N_CORES):
        b, hh = core // 2, core % 2
        oT = np.asarray(results[core]["out"], np.float32)  # [128, DPT*TOKF]
        # oT[p, k*TOKF + t] = out[hh*TOKF + t, b, k*128 + p]
        blk = oT.reshape(128, c.DPT, c.TOKF).transpose(2, 1, 0).reshape(c.TOKF, D)
        out[hh * c.TOKF:(hh + 1) * c.TOKF, b, :] = blk
    return out


_NC_CACHE = {}


def kernel(**inputs):
    if "nc" not in _NC_CACHE:
        _NC_CACHE["nc"] = build_kernel()
    nc = _NC_CACHE["nc"]
    in_maps = shard_inputs(inputs)
    from concourse.bass_utils import run_bass_kernel_spmd
    res = run_bass_kernel_spmd(nc, in_maps, core_ids=list(range(Cfg.N_CORES)))
    return unshard_output(res.results, inputs)
